# revision 3
# baseline (speedup 1.0000x reference)
"""Trainium2 Bass kernel for nn_ACE_77876347011078 (SEAN/SPADE-style block).

Self-contained: hardcodes shapes. Strategy (8 NeuronCores, zero collectives):
  core k -> sample b = k//2, channel half hf = k%2 (128 of 256 channels).
Instance-norm is per-(sample,channel) so channel sharding keeps it local.

Device math per core, all pixels of one sample, 128 output channels:
  - middle_avg is never materialized: conv3x3(middle_avg, W) with a one-hot
    segmap collapses to conv3x3(segmap, R_b) with R_b[o,j,ky,kx] =
    sum_i W[o,i,ky,kx] * mu[b,j,i]  (35 input channels instead of 128).
  - one-hot built on-chip in a "triple" layout [105 = 3*35, cols] so each
    3x3 conv over the segmap is 3 matmuls (K=105) instead of 9 (K=35).
  - SPADE branch: actv = relu(conv(onehot, mlp)) then 9 K=128 matmuls/tap.
  - blending scalars (sigmoid of blending_*) folded into the weights, so
    gamma_final/beta_final each accumulate in a single PSUM group.
  - epilogue: out = ((x - mean) * inv_std) * (gamma + bias1pg) + (beta + biasb)
    via one ScalarE activation + two VectorE scalar_tensor_tensor ops.

Layout: width padded to 257 with one left pad column per row (label -1 ->
zero one-hot = SAME zero padding); a row's right pad is the next row's left
pad. Guard columns at tile ends keep access patterns in bounds.
"""
import numpy as np
import ml_dtypes

B, C, H, W, J, S = 4, 256, 128, 256, 35, 128
P = 128            # channels per core / partition dim
J3 = 105           # 3 * 35 triple one-hot partitions
Wp = W + 1         # left-pad-only padded width
Hc = 32            # strip height
NSTRIP = H // Hc
SPAN_A = (Hc + 2) * Wp      # actv / one-hot center span (8738)
SPAN_C = Hc * Wp            # gamma/beta center span (8224)
NPIX = H * Wp               # padded pixels per core (32896)
NREAL = H * W
EPS = 1e-5
CHUNK = 512
TSTAT = 2056                # 16 stats tiles * 2056 = NPIX

_BF16 = ml_dtypes.bfloat16

_nc_cache = None


def _chunks(total, size):
    out = []
    v = 0
    while v < total:
        n = min(size, total - v)
        out.append((v, n))
        v += n
    return out


def _build_program():
    from concourse import bass, bacc, tile, mybir

    nc = bacc.Bacc("TRN2", target_bir_lowering=False, debug=False, num_devices=8)
    f32 = mybir.dt.float32
    bf16 = mybir.dt.bfloat16
    i8 = mybir.dt.int8

    xp_d = nc.dram_tensor("xp", [P, NPIX], f32, kind="ExternalInput")
    segp_d = nc.dram_tensor("segp", [(H + 4) * Wp], i8, kind="ExternalInput")
    wseg_d = nc.dram_tensor("wseg", [J3, 6 * P], bf16, kind="ExternalInput")
    wmlp_d = nc.dram_tensor("wmlp", [J3, 3 * P], bf16, kind="ExternalInput")
    wsp_d = nc.dram_tensor("wsp", [P, 18 * P], bf16, kind="ExternalInput")
    bias_d = nc.dram_tensor("bias6", [P, 6], f32, kind="ExternalInput")
    iota_d = nc.dram_tensor("iota1", [J3, 1], f32, kind="ExternalInput")
    out_d = nc.dram_tensor("out", [P, NPIX], f32, kind="ExternalOutput")

    AX = mybir.AxisListType.X
    OP = mybir.AluOpType
    AF = mybir.ActivationFunctionType

    with tile.TileContext(nc) as tc:
        with (
            tc.tile_pool(name="const", bufs=1) as constp,
            tc.tile_pool(name="segb", bufs=2) as segbp,
            tc.tile_pool(name="oh", bufs=2) as ohp,
            tc.tile_pool(name="actv", bufs=2) as actvp,
            tc.tile_pool(name="xstat", bufs=2) as xstatp,
            tc.tile_pool(name="xin", bufs=3) as xinp,
            tc.tile_pool(name="blend", bufs=3) as blendp,
            tc.tile_pool(name="outb", bufs=3) as outbp,
            tc.tile_pool(name="ps_a", bufs=2, space="PSUM") as psa,
            tc.tile_pool(name="ps_g", bufs=2, space="PSUM") as psg,
            tc.tile_pool(name="ps_b", bufs=2, space="PSUM") as psb,
        ):
            # ---- constants ----
            w_seg = constp.tile([J3, 6 * P], bf16)
            nc.sync.dma_start(w_seg[:], wseg_d[:])
            w_mlp = constp.tile([J3, 3 * P], bf16)
            nc.sync.dma_start(w_mlp[:], wmlp_d[:])
            w_sp = constp.tile([P, 18 * P], bf16)
            nc.sync.dma_start(w_sp[:], wsp_d[:])
            biases = constp.tile([P, 6], f32)
            nc.sync.dma_start(biases[:], bias_d[:])
            iota_t = constp.tile([J3, 1], f32)
            nc.sync.dma_start(iota_t[:], iota_d[:])
            b_mlp = biases[:, 0:1]
            b_g1p = biases[:, 1:2]
            b_bet = biases[:, 2:3]
            b_eps = biases[:, 3:4]
            b_zero = biases[:, 4:5]

            # ---- instance-norm stats (sum / sumsq over all pixels) ----
            stats = constp.tile([P, 48], f32)
            for t in range(16):
                xt = xstatp.tile([P, TSTAT], f32, tag="xt")
                nc.sync.dma_start(xt[:], xp_d[:, t * TSTAT:(t + 1) * TSTAT])
                sq = xstatp.tile([P, TSTAT], f32, tag="sq")
                nc.scalar.activation(sq[:], xt[:], AF.Square, bias=b_zero)
                nc.vector.reduce_sum(stats[:, t:t + 1], sq[:], axis=AX)
                nc.vector.reduce_sum(stats[:, 16 + t:17 + t], xt[:], axis=AX)
            ssqt = stats[:, 32:33]
            sumt = stats[:, 33:34]
            mean = stats[:, 34:35]
            ex2 = stats[:, 35:36]
            var = stats[:, 36:37]
            sd = stats[:, 37:38]
            inv = stats[:, 38:39]
            biasA = stats[:, 39:40]
            nc.vector.reduce_sum(ssqt, stats[:, 0:16], axis=AX)
            nc.vector.reduce_sum(sumt, stats[:, 16:32], axis=AX)
            nc.vector.tensor_scalar_mul(ex2, ssqt, 1.0 / NREAL)
            nc.vector.tensor_scalar_mul(mean, sumt, 1.0 / NREAL)
            # var = ex2 - mean^2 = (mean * -mean) + ex2
            nc.vector.scalar_tensor_tensor(var, mean, -1.0, mean, OP.mult, OP.mult)
            nc.vector.tensor_add(var, var, ex2)
            nc.scalar.activation(sd, var, AF.Sqrt, bias=b_eps)
            nc.vector.reciprocal(inv, sd)
            # biasA = -mean * inv
            nc.vector.scalar_tensor_tensor(biasA, mean, -1.0, inv, OP.mult, OP.mult)

            # ---- strips ----
            for s in range(NSTRIP):
                h0 = s * Hc
                segb = segbp.tile([J3, SPAN_A], i8)
                for r in range(3):
                    src = bass.AP(segp_d, (h0 + r) * Wp, [[0, 35], [1, SPAN_A]])
                    nc.sync.dma_start(segb[35 * r:35 * r + 35, :], src)

                oh3 = ohp.tile([J3, SPAN_A + 2], bf16)
                nc.vector.memset(oh3[:, 0:1], 0.0)
                nc.vector.memset(oh3[:, SPAN_A + 1:SPAN_A + 2], 0.0)
                nc.vector.tensor_scalar(
                    oh3[:, 1:1 + SPAN_A], segb[:], iota_t[:, 0:1], None,
                    op0=OP.is_equal)

                # actv = relu(mlp conv + b) over centers [h0-1, h0+Hc+1)
                actv = actvp.tile([P, SPAN_A + 2], bf16)
                for v0, n in _chunks(SPAN_A, CHUNK):
                    zp = psa.tile([P, n], mybir.dt.float32, tag="zp")
                    for kx in range(3):
                        nc.tensor.matmul(
                            zp[:], w_mlp[:, kx * P:(kx + 1) * P],
                            oh3[:, v0 + kx:v0 + kx + n],
                            start=(kx == 0), stop=(kx == 2))
                    nc.scalar.activation(
                        actv[:, 1 + v0:1 + v0 + n], zp[:], AF.Relu, bias=b_mlp)
                # zero guard cols, per-row pad col, and out-of-image halo rows
                nc.vector.memset(actv[:, 0:1], 0.0)
                nc.vector.memset(actv[:, 1 + SPAN_A:2 + SPAN_A], 0.0)
                pads = actv[:, 1:1 + SPAN_A].rearrange(
                    "p (a w) -> p a w", w=Wp)[:, :, 0:1]
                nc.vector.memset(pads, 0.0)
                if s == 0:
                    nc.vector.memset(actv[:, 1:1 + Wp], 0.0)
                if s == NSTRIP - 1:
                    nc.vector.memset(actv[:, 1 + (Hc + 1) * Wp:1 + SPAN_A], 0.0)

                # gamma/beta accumulation + blend over centers [h0, h0+Hc)
                for v0, n in _chunks(SPAN_C, CHUNK):
                    gp = psg.tile([P, n], mybir.dt.float32, tag="gp")
                    bp = psb.tile([P, n], mybir.dt.float32, tag="bp")
                    for kx in range(3):
                        rhs = oh3[:, Wp + v0 + kx:Wp + v0 + kx + n]
                        nc.tensor.matmul(
                            gp[:], w_seg[:, (kx * 2) * P:(kx * 2 + 1) * P],
                            rhs, start=(kx == 0), stop=False)
                        nc.tensor.matmul(
                            bp[:], w_seg[:, (kx * 2 + 1) * P:(kx * 2 + 2) * P],
                            rhs, start=(kx == 0), stop=False)
                    for ky in range(3):
                        for kx in range(3):
                            rhs = actv[:, v0 + ky * Wp + kx:v0 + ky * Wp + kx + n]
                            last = (ky == 2 and kx == 2)
                            t0 = ((ky * 3 + kx) * 2) * P
                            t1 = ((ky * 3 + kx) * 2 + 1) * P
                            nc.tensor.matmul(
                                gp[:], w_sp[:, t0:t0 + P], rhs,
                                start=False, stop=last)
                            nc.tensor.matmul(
                                bp[:], w_sp[:, t1:t1 + P], rhs,
                                start=False, stop=last)
                    xt = xinp.tile([P, n], mybir.dt.float32, tag="xin")
                    nc.sync.dma_start(
                        xt[:], xp_d[:, h0 * Wp + v0:h0 * Wp + v0 + n])
                    nt = blendp.tile([P, n], mybir.dt.float32, tag="norm")
                    nc.scalar.activation(
                        nt[:], xt[:], AF.Identity, bias=biasA, scale=inv)
                    t1t = blendp.tile([P, n], mybir.dt.float32, tag="t1")
                    nc.vector.scalar_tensor_tensor(
                        t1t[:], gp[:], b_g1p, nt[:], OP.add, OP.mult)
                    ot = outbp.tile([P, n], mybir.dt.float32, tag="ot")
                    nc.vector.scalar_tensor_tensor(
                        ot[:], bp[:], b_bet, t1t[:], OP.add, OP.add)
                    nc.sync.dma_start(
                        out_d[:, h0 * Wp + v0:h0 * Wp + v0 + n], ot[:])

    nc.compile()
    return nc


def _host_prep(inputs):
    x = np.asarray(inputs["x"], np.float32)
    seg = np.asarray(inputs["seg_labels"]).astype(np.int32)
    ga = float(1.0 / (1.0 + np.exp(-np.asarray(inputs["blending_gamma"], np.float64)[0])))
    ba = float(1.0 / (1.0 + np.exp(-np.asarray(inputs["blending_beta"], np.float64)[0])))
    fc_w = np.asarray(inputs["fc_w"], np.float32)
    fc_b = np.asarray(inputs["fc_b"], np.float32)
    style = np.asarray(inputs["style_codes"], np.float32)
    mu = np.maximum(np.einsum("bjd,jod->bjo", style, fc_w) + fc_b[None], 0.0)
    Rg = np.einsum("oiyx,bji->bojyx", np.asarray(inputs["conv_gamma_w"], np.float32), mu) * ga
    Rb = np.einsum("oiyx,bji->bojyx", np.asarray(inputs["conv_beta_w"], np.float32), mu) * ba
    mlp_w = np.asarray(inputs["mlp_w"], np.float32)
    iota = (np.arange(J3) % 35).astype(np.float32)[:, None]

    in_maps = []
    for core in range(8):
        b, hf = core // 2, core % 2
        sl = slice(hf * P, (hf + 1) * P)
        wseg = np.zeros((J3, 6 * P), np.float32)
        wmlp = np.zeros((J3, 3 * P), np.float32)
        for ky in range(3):
            for kx in range(3):
                rows = slice(35 * ky, 35 * ky + 35)
                wseg[rows, (kx * 2) * P:(kx * 2 + 1) * P] = Rg[b, sl, :, ky, kx].T
                wseg[rows, (kx * 2 + 1) * P:(kx * 2 + 2) * P] = Rb[b, sl, :, ky, kx].T
                wmlp[rows, kx * P:(kx + 1) * P] = mlp_w[:, :, ky, kx].T
        wsp = np.zeros((P, 18 * P), np.float32)
        spg = (1 - ga) * np.asarray(inputs["sp_gamma_w"], np.float32)[sl]
        spb = (1 - ba) * np.asarray(inputs["sp_beta_w"], np.float32)[sl]
        for ky in range(3):
            for kx in range(3):
                t0 = ((ky * 3 + kx) * 2) * P
                t1 = ((ky * 3 + kx) * 2 + 1) * P
                wsp[:, t0:t0 + P] = spg[:, :, ky, kx].T
                wsp[:, t1:t1 + P] = spb[:, :, ky, kx].T
        bias6 = np.zeros((P, 6), np.float32)
        bias6[:, 0] = np.asarray(inputs["mlp_b"], np.float32)
        bias6[:, 1] = (1.0 + ga * np.asarray(inputs["conv_gamma_b"], np.float32)[sl]
                       + (1 - ga) * np.asarray(inputs["sp_gamma_b"], np.float32)[sl])
        bias6[:, 2] = (ba * np.asarray(inputs["conv_beta_b"], np.float32)[sl]
                       + (1 - ba) * np.asarray(inputs["sp_beta_b"], np.float32)[sl])
        bias6[:, 3] = EPS
        seg_pad = np.full((H + 4, Wp), -1, np.int8)
        seg_pad[2:2 + H, 1:] = seg[b]
        x_pad = np.zeros((P, H, Wp), np.float32)
        x_pad[:, :, 1:] = x[b, sl]
        in_maps.append({
            "xp": np.ascontiguousarray(x_pad.reshape(P, NPIX)),
            "segp": seg_pad.reshape(-1),
            "wseg": wseg.astype(_BF16),
            "wmlp": wmlp.astype(_BF16),
            "wsp": wsp.astype(_BF16),
            "bias6": bias6,
            "iota1": iota,
        })
    return in_maps


def get_nc():
    global _nc_cache
    if _nc_cache is None:
        _nc_cache = _build_program()
    return _nc_cache


def kernel(**inputs):
    from concourse.bass_utils import run_bass_kernel_spmd

    in_maps = _host_prep(inputs)
    nc = get_nc()
    res = run_bass_kernel_spmd(nc, in_maps, core_ids=list(range(8)))
    out = np.empty((B, C, H, W), np.float32)
    for core in range(8):
        b, hf = core // 2, core % 2
        op = res.results[core]["out"].reshape(P, H, Wp)
        out[b, hf * P:(hf + 1) * P] = op[:, :, 1:]
    return out


# revision 10
# speedup vs baseline: 1.1059x; 1.1059x over previous
"""Trainium2 Bass kernel for nn_ACE_77876347011078 (SEAN/SPADE-style block).

Self-contained: hardcodes shapes. Strategy (8 NeuronCores, zero collectives):
  core k -> sample b = k//2, channel half hf = k%2 (128 of 256 channels).
Instance-norm is per-(sample,channel) so channel sharding keeps it local.

Device math per core, all pixels of one sample, 128 output channels:
  - middle_avg is never materialized: conv3x3(middle_avg, W) with a one-hot
    segmap collapses to conv3x3(segmap, R_b) with R_b[o,j,ky,kx] =
    sum_i W[o,i,ky,kx] * mu[b,j,i]  (35 input channels instead of 128).
  - one-hot built on-chip in a "triple" layout [105 = 3*35, cols] so each
    3x3 conv over the segmap is 3 matmuls (K=105) instead of 9 (K=35).
  - SPADE branch: actv = relu(conv(onehot, mlp)) then 9 K=128 matmuls/tap.
  - blending scalars (sigmoid of blending_*) folded into the weights, so
    gamma_final/beta_final each accumulate in a single PSUM group.
  - epilogue: out = ((x - mean) * inv_std) * (gamma + bias1pg) + (beta + biasb)
    via one ScalarE activation + two VectorE scalar_tensor_tensor ops.

Layout: width padded to 257 with one left pad column per row (label -1 ->
zero one-hot = SAME zero padding); a row's right pad is the next row's left
pad. Guard columns at tile ends keep access patterns in bounds.
"""
import numpy as np
import ml_dtypes

B, C, H, W, J, S = 4, 256, 128, 256, 35, 128
P = 128            # channels per core / partition dim
J3 = 105           # 3 * 35 triple one-hot partitions
Wp = W + 1         # left-pad-only padded width
Hc = 32            # strip height
NSTRIP = H // Hc
SPAN_A = (Hc + 2) * Wp      # actv / one-hot center span (8738)
SPAN_C = Hc * Wp            # gamma/beta center span (8224)
NPIX = H * Wp               # padded pixels per core (32896)
NREAL = H * W
EPS = 1e-5
CHUNK = 512
TSTAT = 2056                # 16 stats tiles * 2056 = NPIX

_BF16 = ml_dtypes.bfloat16

_nc_cache = None


def _chunks(total, size):
    out = []
    v = 0
    while v < total:
        n = min(size, total - v)
        out.append((v, n))
        v += n
    return out


def _build_program():
    from concourse import bass, bacc, tile, mybir

    nc = bacc.Bacc("TRN2", target_bir_lowering=False, debug=False, num_devices=8)
    f32 = mybir.dt.float32
    bf16 = mybir.dt.bfloat16
    i8 = mybir.dt.int8

    fp8 = mybir.dt.float8e4

    xp_d = nc.dram_tensor("xp", [P, NPIX], f32, kind="ExternalInput")
    segp_d = nc.dram_tensor("segp", [(H + 4) * Wp], i8, kind="ExternalInput")
    wseg_d = nc.dram_tensor("wseg", [J3, 6 * P], bf16, kind="ExternalInput")
    wmlp_d = nc.dram_tensor("wmlp", [J3, 3 * P], bf16, kind="ExternalInput")
    wsp_d = nc.dram_tensor("wsp", [P, 18 * P], fp8, kind="ExternalInput")
    bias_d = nc.dram_tensor("bias6", [P, 6], f32, kind="ExternalInput")
    iota_d = nc.dram_tensor("iota1", [J3, 1], f32, kind="ExternalInput")
    out_d = nc.dram_tensor("out", [P, NPIX], f32, kind="ExternalOutput")

    AX = mybir.AxisListType.X
    OP = mybir.AluOpType
    AF = mybir.ActivationFunctionType

    with tile.TileContext(nc) as tc:
        with (
            tc.tile_pool(name="const", bufs=1) as constp,
            tc.tile_pool(name="segb", bufs=2) as segbp,
            tc.tile_pool(name="oh", bufs=2) as ohp,
            tc.tile_pool(name="actv", bufs=2) as actvp,
            tc.tile_pool(name="xstat", bufs=2) as xstatp,
            tc.tile_pool(name="xin", bufs=3) as xinp,
            tc.tile_pool(name="blend", bufs=3) as blendp,
            tc.tile_pool(name="outb", bufs=3) as outbp,
            tc.tile_pool(name="ps_a", bufs=2, space="PSUM") as psa,
            tc.tile_pool(name="ps_g", bufs=3, space="PSUM") as psg,
            tc.tile_pool(name="ps_b", bufs=3, space="PSUM") as psb,
        ):
            # ---- constants ----
            w_seg = constp.tile([J3, 6 * P], bf16)
            nc.sync.dma_start(w_seg[:], wseg_d[:])
            w_mlp = constp.tile([J3, 3 * P], bf16)
            nc.sync.dma_start(w_mlp[:], wmlp_d[:])
            w_sp = constp.tile([P, 18 * P], fp8)
            nc.sync.dma_start(w_sp[:], wsp_d[:])
            biases = constp.tile([P, 6], f32)
            nc.sync.dma_start(biases[:], bias_d[:])
            iota_t = constp.tile([J3, 1], f32)
            nc.sync.dma_start(iota_t[:], iota_d[:])
            b_mlp = biases[:, 0:1]
            b_g1p = biases[:, 1:2]
            b_bet = biases[:, 2:3]
            b_eps = biases[:, 3:4]
            b_zero = biases[:, 4:5]

            # ---- instance-norm stats (sum / sumsq over all pixels) ----
            stats = constp.tile([P, 48], f32)
            for t in range(16):
                xt = xstatp.tile([P, TSTAT], f32, tag="xt")
                nc.sync.dma_start(xt[:], xp_d[:, t * TSTAT:(t + 1) * TSTAT])
                sq = xstatp.tile([P, TSTAT], f32, tag="sq")
                nc.scalar.activation(sq[:], xt[:], AF.Square, bias=b_zero)
                nc.vector.reduce_sum(stats[:, t:t + 1], sq[:], axis=AX)
                nc.vector.reduce_sum(stats[:, 16 + t:17 + t], xt[:], axis=AX)
            ssqt = stats[:, 32:33]
            sumt = stats[:, 33:34]
            mean = stats[:, 34:35]
            ex2 = stats[:, 35:36]
            var = stats[:, 36:37]
            sd = stats[:, 37:38]
            inv = stats[:, 38:39]
            biasA = stats[:, 39:40]
            nc.vector.reduce_sum(ssqt, stats[:, 0:16], axis=AX)
            nc.vector.reduce_sum(sumt, stats[:, 16:32], axis=AX)
            nc.vector.tensor_scalar_mul(ex2, ssqt, 1.0 / NREAL)
            nc.vector.tensor_scalar_mul(mean, sumt, 1.0 / NREAL)
            # var = ex2 - mean^2 = (mean * -mean) + ex2
            nc.vector.scalar_tensor_tensor(var, mean, -1.0, mean, OP.mult, OP.mult)
            nc.vector.tensor_add(var, var, ex2)
            nc.scalar.activation(sd, var, AF.Sqrt, bias=b_eps)
            nc.vector.reciprocal(inv, sd)
            # biasA = -mean * inv
            nc.vector.scalar_tensor_tensor(biasA, mean, -1.0, inv, OP.mult, OP.mult)

            # ---- strips ----
            for s in range(NSTRIP):
                h0 = s * Hc
                segb = segbp.tile([J3, SPAN_A], i8)
                for r in range(3):
                    src = bass.AP(segp_d, (h0 + r) * Wp, [[0, 35], [1, SPAN_A]])
                    nc.sync.dma_start(segb[35 * r:35 * r + 35, :], src)

                oh3 = ohp.tile([J3, SPAN_A + 2], bf16)
                nc.vector.memset(oh3[:, 0:1], 0.0)
                nc.vector.memset(oh3[:, SPAN_A + 1:SPAN_A + 2], 0.0)
                nc.vector.tensor_scalar(
                    oh3[:, 1:1 + SPAN_A], segb[:], iota_t[:, 0:1], None,
                    op0=OP.is_equal)

                # actv = relu(mlp conv + b) over centers [h0-1, h0+Hc+1)
                # stored fp8 (feeds the fp8 DoubleRow SPADE matmuls)
                actv = actvp.tile([P, SPAN_A + 2], fp8)
                for v0, n in _chunks(SPAN_A, CHUNK):
                    zp = psa.tile([P, n], mybir.dt.float32, tag="zp")
                    for kx in range(3):
                        nc.tensor.matmul(
                            zp[:], w_mlp[:, kx * P:(kx + 1) * P],
                            oh3[:, v0 + kx:v0 + kx + n],
                            start=(kx == 0), stop=(kx == 2))
                    nc.scalar.activation(
                        actv[:, 1 + v0:1 + v0 + n], zp[:], AF.Relu, bias=b_mlp)
                # zero guard cols, per-row pad col, and out-of-image halo rows
                nc.vector.memset(actv[:, 0:1], 0.0)
                nc.vector.memset(actv[:, 1 + SPAN_A:2 + SPAN_A], 0.0)
                pads = actv[:, 1:1 + SPAN_A].rearrange(
                    "p (a w) -> p a w", w=Wp)[:, :, 0:1]
                nc.vector.memset(pads, 0.0)
                if s == 0:
                    nc.vector.memset(actv[:, 1:1 + Wp], 0.0)
                if s == NSTRIP - 1:
                    nc.vector.memset(actv[:, 1 + (Hc + 1) * Wp:1 + SPAN_A], 0.0)

                # gamma/beta accumulation + blend over centers [h0, h0+Hc)
                for v0, n in _chunks(SPAN_C, CHUNK):
                    gp = psg.tile([P, n], mybir.dt.float32, tag="gp")
                    bp = psb.tile([P, n], mybir.dt.float32, tag="bp")
                    for kx in range(3):
                        rhs = oh3[:, Wp + v0 + kx:Wp + v0 + kx + n]
                        nc.tensor.matmul(
                            gp[:], w_seg[:, (kx * 2) * P:(kx * 2 + 1) * P],
                            rhs, start=(kx == 0), stop=False)
                        nc.tensor.matmul(
                            bp[:], w_seg[:, (kx * 2 + 1) * P:(kx * 2 + 2) * P],
                            rhs, start=(kx == 0), stop=False)
                    # SPADE convs: fp8, 4 DoubleRow pairs + 1 plain per side.
                    # pair groups: (tap offsets rel. to v0, rhs pair step)
                    a_ap = actv[:]
                    pitch = a_ap.ap[0][0]

                    def rhs_pair(base, step, nn):
                        return bass.AP(a_ap.tensor, base,
                                       [[pitch, P], [step, 2], [1, nn]])
                    DRG = [(0, 1), (Wp, 1), (2 * Wp, 1), (2, Wp)]
                    for grp, (ofs, step) in enumerate(DRG):
                        rhs = rhs_pair(v0 + ofs, step, n)
                        for t, pt in ((0, gp), (1, bp)):
                            wofs = (grp * 2) * P if t == 0 else (9 + grp * 2) * P
                            lw = w_sp[:, wofs:wofs + 2 * P].rearrange(
                                "p (t m) -> p t m", m=P)
                            nc.tensor.matmul(
                                pt[:], lw, rhs, start=False, stop=False,
                                perf_mode=mybir.MatmulPerfMode.DoubleRow)
                    # single tap (2,2)
                    rhs = actv[:, v0 + 2 * Wp + 2:v0 + 2 * Wp + 2 + n]
                    nc.tensor.matmul(
                        gp[:], w_sp[:, 8 * P:9 * P], rhs,
                        start=False, stop=True)
                    nc.tensor.matmul(
                        bp[:], w_sp[:, 17 * P:18 * P], rhs,
                        start=False, stop=True)
                    xt = xinp.tile([P, n], mybir.dt.float32, tag="xin")
                    nc.sync.dma_start(
                        xt[:], xp_d[:, h0 * Wp + v0:h0 * Wp + v0 + n])
                    nt = blendp.tile([P, n], mybir.dt.float32, tag="norm")
                    nc.scalar.activation(
                        nt[:], xt[:], AF.Identity, bias=biasA, scale=inv)
                    t1t = blendp.tile([P, n], mybir.dt.float32, tag="t1")
                    nc.vector.scalar_tensor_tensor(
                        t1t[:], gp[:], b_g1p, nt[:], OP.add, OP.mult)
                    ot = outbp.tile([P, n], mybir.dt.float32, tag="ot")
                    nc.vector.scalar_tensor_tensor(
                        ot[:], bp[:], b_bet, t1t[:], OP.add, OP.add)
                    nc.sync.dma_start(
                        out_d[:, h0 * Wp + v0:h0 * Wp + v0 + n], ot[:])

    nc.compile()
    return nc


def _host_prep(inputs):
    x = np.asarray(inputs["x"], np.float32)
    seg = np.asarray(inputs["seg_labels"]).astype(np.int32)
    ga = float(1.0 / (1.0 + np.exp(-np.asarray(inputs["blending_gamma"], np.float64)[0])))
    ba = float(1.0 / (1.0 + np.exp(-np.asarray(inputs["blending_beta"], np.float64)[0])))
    fc_w = np.asarray(inputs["fc_w"], np.float32)
    fc_b = np.asarray(inputs["fc_b"], np.float32)
    style = np.asarray(inputs["style_codes"], np.float32)
    mu = np.maximum(np.einsum("bjd,jod->bjo", style, fc_w) + fc_b[None], 0.0)
    Rg = np.einsum("oiyx,bji->bojyx", np.asarray(inputs["conv_gamma_w"], np.float32), mu) * ga
    Rb = np.einsum("oiyx,bji->bojyx", np.asarray(inputs["conv_beta_w"], np.float32), mu) * ba
    mlp_w = np.asarray(inputs["mlp_w"], np.float32)
    iota = (np.arange(J3) % 35).astype(np.float32)[:, None]

    in_maps = []
    for core in range(8):
        b, hf = core // 2, core % 2
        sl = slice(hf * P, (hf + 1) * P)
        wseg = np.zeros((J3, 6 * P), np.float32)
        wmlp = np.zeros((J3, 3 * P), np.float32)
        for ky in range(3):
            for kx in range(3):
                rows = slice(35 * ky, 35 * ky + 35)
                wseg[rows, (kx * 2) * P:(kx * 2 + 1) * P] = Rg[b, sl, :, ky, kx].T
                wseg[rows, (kx * 2 + 1) * P:(kx * 2 + 2) * P] = Rb[b, sl, :, ky, kx].T
                wmlp[rows, kx * P:(kx + 1) * P] = mlp_w[:, :, ky, kx].T
        # fp8 DoubleRow layout: gamma pairs G0..G3 at [g*2P, g*2P+2P)
        # (cols t*P+o for the pair's two taps), single (2,2) at [8P,9P);
        # beta mirrors at +9P.
        wsp = np.zeros((P, 18 * P), np.float32)
        spg = (1 - ga) * np.asarray(inputs["sp_gamma_w"], np.float32)[sl]
        spb = (1 - ba) * np.asarray(inputs["sp_beta_w"], np.float32)[sl]
        PAIRS = [[(0, 0), (0, 1)], [(1, 0), (1, 1)], [(2, 0), (2, 1)],
                 [(0, 2), (1, 2)]]
        for g, pair in enumerate(PAIRS):
            for t, (ky, kx) in enumerate(pair):
                wsp[:, (g * 2 + t) * P:(g * 2 + t + 1) * P] = spg[:, :, ky, kx].T
                wsp[:, (9 + g * 2 + t) * P:(9 + g * 2 + t + 1) * P] = spb[:, :, ky, kx].T
        wsp[:, 8 * P:9 * P] = spg[:, :, 2, 2].T
        wsp[:, 17 * P:18 * P] = spb[:, :, 2, 2].T
        bias6 = np.zeros((P, 6), np.float32)
        bias6[:, 0] = np.asarray(inputs["mlp_b"], np.float32)
        bias6[:, 1] = (1.0 + ga * np.asarray(inputs["conv_gamma_b"], np.float32)[sl]
                       + (1 - ga) * np.asarray(inputs["sp_gamma_b"], np.float32)[sl])
        bias6[:, 2] = (ba * np.asarray(inputs["conv_beta_b"], np.float32)[sl]
                       + (1 - ba) * np.asarray(inputs["sp_beta_b"], np.float32)[sl])
        bias6[:, 3] = EPS
        seg_pad = np.full((H + 4, Wp), -1, np.int8)
        seg_pad[2:2 + H, 1:] = seg[b]
        x_pad = np.zeros((P, H, Wp), np.float32)
        x_pad[:, :, 1:] = x[b, sl]
        in_maps.append({
            "xp": np.ascontiguousarray(x_pad.reshape(P, NPIX)),
            "segp": seg_pad.reshape(-1),
            "wseg": wseg.astype(_BF16),
            "wmlp": wmlp.astype(_BF16),
            "wsp": wsp.astype(ml_dtypes.float8_e4m3),
            "bias6": bias6,
            "iota1": iota,
        })
    return in_maps


def get_nc():
    global _nc_cache
    if _nc_cache is None:
        _nc_cache = _build_program()
    return _nc_cache


def kernel(**inputs):
    from concourse.bass_utils import run_bass_kernel_spmd

    in_maps = _host_prep(inputs)
    nc = get_nc()
    res = run_bass_kernel_spmd(nc, in_maps, core_ids=list(range(8)))
    out = np.empty((B, C, H, W), np.float32)
    for core in range(8):
        b, hf = core // 2, core % 2
        op = res.results[core]["out"].reshape(P, H, Wp)
        out[b, hf * P:(hf + 1) * P] = op[:, :, 1:]
    return out


# revision 18
# speedup vs baseline: 1.3396x; 1.2113x over previous
"""Trainium2 Bass kernel for nn_ACE_77876347011078 (SEAN/SPADE-style block).

Self-contained: hardcodes shapes. Strategy (8 NeuronCores, zero collectives):
  core k -> sample b = k//2, channel half hf = k%2 (128 of 256 channels).
Instance-norm is per-(sample,channel) so channel sharding keeps it local.

Device math per core, all pixels of one sample, 128 output channels:
  - middle_avg is never materialized: conv3x3(middle_avg, W) with a one-hot
    segmap collapses to conv3x3(segmap, R_b) with R_b[o,j,ky,kx] =
    sum_i W[o,i,ky,kx] * mu[b,j,i]  (35 input channels instead of 128).
  - one-hot built on-chip in a "triple" layout [105 = 3*35, cols] so each
    3x3 conv over the segmap is 3 matmuls (K=105) instead of 9 (K=35).
  - SPADE branch: actv = relu(conv(onehot, mlp)) then 9 K=128 matmuls/tap.
  - blending scalars (sigmoid of blending_*) folded into the weights, so
    gamma_final/beta_final each accumulate in a single PSUM group.
  - epilogue: out = ((x - mean) * inv_std) * (gamma + bias1pg) + (beta + biasb)
    via one ScalarE activation + two VectorE scalar_tensor_tensor ops.

Layout: width padded to 257 with one left pad column per row (label -1 ->
zero one-hot = SAME zero padding); a row's right pad is the next row's left
pad. Guard columns at tile ends keep access patterns in bounds.
"""
import numpy as np
import ml_dtypes

B, C, H, W, J, S = 4, 256, 128, 256, 35, 128
P = 128            # channels per core / partition dim
J3 = 105           # 3 * 35 triple one-hot partitions
Wp = W + 1         # left-pad-only padded width
Hc = 32            # strip height
NSTRIP = H // Hc
SPAN_A = (Hc + 2) * Wp      # actv / one-hot center span (8738)
SPAN_C = Hc * Wp            # gamma/beta center span (8224)
NPIX = H * Wp               # padded pixels per core (32896)
NREAL = H * W
EPS = 1e-5
CHUNK = 512
TSTAT = 4112                # 8 stats tiles * 4112 = NPIX

_BF16 = ml_dtypes.bfloat16

_nc_cache = None


def _chunks(total, size):
    out = []
    v = 0
    while v < total:
        n = min(size, total - v)
        out.append((v, n))
        v += n
    return out


def _build_program():
    from concourse import bass, bacc, tile, mybir

    nc = bacc.Bacc("TRN2", target_bir_lowering=False, debug=False, num_devices=8)
    f32 = mybir.dt.float32
    bf16 = mybir.dt.bfloat16
    i8 = mybir.dt.int8

    fp8 = mybir.dt.float8e4

    xp_d = nc.dram_tensor("xp", [P, NPIX], f32, kind="ExternalInput")
    segp_d = nc.dram_tensor("segp", [(H + 4) * Wp], i8, kind="ExternalInput")
    wseg_d = nc.dram_tensor("wseg", [J3, 6 * P], bf16, kind="ExternalInput")
    wmlp_d = nc.dram_tensor("wmlp", [J3, 3 * P], fp8, kind="ExternalInput")
    wsp_d = nc.dram_tensor("wsp", [P, 18 * P], fp8, kind="ExternalInput")
    bias_d = nc.dram_tensor("bias6", [P, 6], f32, kind="ExternalInput")
    iota_d = nc.dram_tensor("iota1", [J3, 1], f32, kind="ExternalInput")
    out_d = nc.dram_tensor("out", [P, NPIX], f32, kind="ExternalOutput")

    AX = mybir.AxisListType.X
    OP = mybir.AluOpType
    AF = mybir.ActivationFunctionType

    with tile.TileContext(nc) as tc:
        with (
            tc.tile_pool(name="const", bufs=1) as constp,
            tc.tile_pool(name="segb", bufs=2) as segbp,
            tc.tile_pool(name="oh", bufs=2) as ohp,
            tc.tile_pool(name="actv", bufs=2) as actvp,
            tc.tile_pool(name="xstat", bufs=2) as xstatp,
            tc.tile_pool(name="xin", bufs=3) as xinp,
            tc.tile_pool(name="blend", bufs=3) as blendp,
            tc.tile_pool(name="outb", bufs=3) as outbp,
            tc.tile_pool(name="ps_a", bufs=2, space="PSUM") as psa,
            tc.tile_pool(name="ps_g", bufs=3, space="PSUM") as psg,
            tc.tile_pool(name="ps_b", bufs=3, space="PSUM") as psb,
        ):
            # ---- constants ----
            w_seg = constp.tile([J3, 6 * P], bf16)
            nc.sync.dma_start(w_seg[:], wseg_d[:])
            w_mlp = constp.tile([J3, 3 * P], fp8)
            nc.sync.dma_start(w_mlp[:], wmlp_d[:])
            w_sp = constp.tile([P, 18 * P], fp8)
            nc.sync.dma_start(w_sp[:], wsp_d[:])
            biases = constp.tile([P, 6], f32)
            nc.sync.dma_start(biases[:], bias_d[:])
            iota_t = constp.tile([J3, 1], f32)
            nc.sync.dma_start(iota_t[:], iota_d[:])
            b_mlp = biases[:, 0:1]
            b_g1p = biases[:, 1:2]
            b_bet = biases[:, 2:3]
            b_eps = biases[:, 3:4]
            b_zero = biases[:, 4:5]

            # ---- instance-norm stats (sum / sumsq over all pixels) ----
            # sumsq on ScalarE via activation accum_out, sum on VectorE, in
            # parallel -- keeps the stats critical path ~DMA-bound.
            stats = constp.tile([P, 48], f32)
            for t in range(8):
                xt = xstatp.tile([P, TSTAT], f32, tag="xt")
                nc.sync.dma_start(xt[:], xp_d[:, t * TSTAT:(t + 1) * TSTAT])
                sq = xstatp.tile([P, TSTAT], f32, tag="sq")
                nc.scalar.activation(sq[:], xt[:], AF.Square, bias=b_zero,
                                     accum_out=stats[:, t:t + 1])
                nc.vector.reduce_sum(stats[:, 8 + t:9 + t], xt[:], axis=AX)
            ssqt = stats[:, 32:33]
            sumt = stats[:, 33:34]
            mean = stats[:, 34:35]
            ex2 = stats[:, 35:36]
            var = stats[:, 36:37]
            sd = stats[:, 37:38]
            inv = stats[:, 38:39]
            biasA = stats[:, 39:40]
            nc.vector.reduce_sum(ssqt, stats[:, 0:8], axis=AX)
            nc.vector.reduce_sum(sumt, stats[:, 8:16], axis=AX)
            nc.vector.tensor_scalar_mul(ex2, ssqt, 1.0 / NREAL)
            nc.vector.tensor_scalar_mul(mean, sumt, 1.0 / NREAL)
            # var = ex2 - mean^2 = (mean * -mean) + ex2
            nc.vector.scalar_tensor_tensor(var, mean, -1.0, mean, OP.mult, OP.mult)
            nc.vector.tensor_add(var, var, ex2)
            nc.scalar.activation(sd, var, AF.Sqrt, bias=b_eps)
            nc.vector.reciprocal(inv, sd)
            # biasA = -mean * inv
            nc.vector.scalar_tensor_tensor(biasA, mean, -1.0, inv, OP.mult, OP.mult)

            # ---- strips ----
            for s in range(NSTRIP):
                h0 = s * Hc
                segb = segbp.tile([J3, SPAN_A], i8)
                for r in range(3):
                    src = bass.AP(segp_d, (h0 + r) * Wp, [[0, 35], [1, SPAN_A]])
                    nc.sync.dma_start(segb[35 * r:35 * r + 35, :], src)

                # bf16 one-hot feeds the (accuracy-sensitive) segmap convs;
                # fp8 one-hot feeds the DoubleRow mlp conv. 0/1 exact in both.
                oh3 = ohp.tile([J3, SPAN_A + 2], bf16, tag="oh3")
                nc.vector.memset(oh3[:, 0:1], 0.0)
                nc.vector.memset(oh3[:, SPAN_A + 1:SPAN_A + 2], 0.0)
                nc.vector.tensor_scalar(
                    oh3[:, 1:1 + SPAN_A], segb[:], iota_t[:, 0:1], None,
                    op0=OP.is_equal)
                oh3f = ohp.tile([J3, SPAN_A + 2], fp8, tag="oh3f")
                nc.vector.memset(oh3f[:, 0:1], 0.0)
                nc.vector.memset(oh3f[:, SPAN_A + 1:SPAN_A + 2], 0.0)
                nc.vector.tensor_scalar(
                    oh3f[:, 1:1 + SPAN_A], segb[:], iota_t[:, 0:1], None,
                    op0=OP.is_equal)

                of_ap = oh3f[:]
                of_pitch = of_ap.ap[0][0]

                # actv = relu(mlp conv + b) over centers [h0-1, h0+Hc+1)
                # fp8 DoubleRow pair (kx=0,1) + single kx=2; stored fp8
                actv = actvp.tile([P, SPAN_A + 2], fp8)
                lw_mlp_dr = w_mlp[:, 0:2 * P].rearrange("p (t m) -> p t m", m=P)
                for v0, n in _chunks(SPAN_A, CHUNK):
                    zp = psa.tile([P, n], mybir.dt.float32, tag="zp")
                    rhs = bass.AP(of_ap.tensor, v0,
                                  [[of_pitch, J3], [1, 2], [1, n]])
                    nc.tensor.matmul(
                        zp[:], lw_mlp_dr, rhs, start=True, stop=False,
                        perf_mode=mybir.MatmulPerfMode.DoubleRow)
                    nc.tensor.matmul(
                        zp[:], w_mlp[:, 2 * P:3 * P],
                        oh3f[:, v0 + 2:v0 + 2 + n], start=False, stop=True)
                    nc.scalar.activation(
                        actv[:, 1 + v0:1 + v0 + n], zp[:], AF.Relu, bias=b_mlp)
                # zero guard cols, per-row pad col, and out-of-image halo rows
                nc.vector.memset(actv[:, 0:1], 0.0)
                nc.vector.memset(actv[:, 1 + SPAN_A:2 + SPAN_A], 0.0)
                pads = actv[:, 1:1 + SPAN_A].rearrange(
                    "p (a w) -> p a w", w=Wp)[:, :, 0:1]
                nc.vector.memset(pads, 0.0)
                if s == 0:
                    nc.vector.memset(actv[:, 1:1 + Wp], 0.0)
                if s == NSTRIP - 1:
                    nc.vector.memset(actv[:, 1 + (Hc + 1) * Wp:1 + SPAN_A], 0.0)

                # gamma/beta accumulation + blend over centers [h0, h0+Hc).
                # chunks processed in pairs, weight-major, so consecutive
                # matmuls share the stationary operand.
                a_ap = actv[:]
                a_pitch = a_ap.ap[0][0]
                DRG = [(0, 1), (Wp, 1), (2 * Wp, 1), (2, Wp)]
                clist = _chunks(SPAN_C, CHUNK)
                groups = [clist[i:i + 2] for i in range(0, len(clist), 2)]
                for grp_chunks in groups:
                    gps = [psg.tile([P, n], mybir.dt.float32, tag="gp",
                                    name=f"gp_{s}_{v0}")
                           for v0, n in grp_chunks]
                    bps = [psb.tile([P, n], mybir.dt.float32, tag="bp",
                                    name=f"bp_{s}_{v0}")
                           for v0, n in grp_chunks]
                    # segmap convs (bf16, K=105)
                    for kx in range(3):
                        lwg = w_seg[:, (kx * 2) * P:(kx * 2 + 1) * P]
                        for c, (v0, n) in enumerate(grp_chunks):
                            nc.tensor.matmul(
                                gps[c][:], lwg,
                                oh3[:, Wp + v0 + kx:Wp + v0 + kx + n],
                                start=(kx == 0), stop=False)
                        lwb = w_seg[:, (kx * 2 + 1) * P:(kx * 2 + 2) * P]
                        for c, (v0, n) in enumerate(grp_chunks):
                            nc.tensor.matmul(
                                bps[c][:], lwb,
                                oh3[:, Wp + v0 + kx:Wp + v0 + kx + n],
                                start=(kx == 0), stop=False)
                    # SPADE convs: fp8, 4 DoubleRow pairs + 1 plain per side
                    for g, (ofs, step) in enumerate(DRG):
                        for t, pts in ((0, gps), (1, bps)):
                            wofs = (g * 2) * P if t == 0 else (9 + g * 2) * P
                            lw = w_sp[:, wofs:wofs + 2 * P].rearrange(
                                "p (t m) -> p t m", m=P)
                            for c, (v0, n) in enumerate(grp_chunks):
                                rhs = bass.AP(a_ap.tensor, v0 + ofs,
                                              [[a_pitch, P], [step, 2], [1, n]])
                                nc.tensor.matmul(
                                    pts[c][:], lw, rhs, start=False, stop=False,
                                    perf_mode=mybir.MatmulPerfMode.DoubleRow)
                    # single tap (2,2)
                    for c, (v0, n) in enumerate(grp_chunks):
                        rhs = actv[:, v0 + 2 * Wp + 2:v0 + 2 * Wp + 2 + n]
                        nc.tensor.matmul(
                            gps[c][:], w_sp[:, 8 * P:9 * P], rhs,
                            start=False, stop=True)
                    for c, (v0, n) in enumerate(grp_chunks):
                        rhs = actv[:, v0 + 2 * Wp + 2:v0 + 2 * Wp + 2 + n]
                        nc.tensor.matmul(
                            bps[c][:], w_sp[:, 17 * P:18 * P], rhs,
                            start=False, stop=True)
                    # blend epilogue per chunk
                    for c, (v0, n) in enumerate(grp_chunks):
                        xt = xinp.tile([P, n], mybir.dt.float32, tag="xin")
                        nc.sync.dma_start(
                            xt[:], xp_d[:, h0 * Wp + v0:h0 * Wp + v0 + n])
                        nt = blendp.tile([P, n], mybir.dt.float32, tag="norm")
                        nc.scalar.activation(
                            nt[:], xt[:], AF.Identity, bias=biasA, scale=inv)
                        t1t = blendp.tile([P, n], mybir.dt.float32, tag="t1")
                        nc.vector.scalar_tensor_tensor(
                            t1t[:], gps[c][:], b_g1p, nt[:], OP.add, OP.mult)
                        ot = outbp.tile([P, n], mybir.dt.float32, tag="ot")
                        nc.vector.scalar_tensor_tensor(
                            ot[:], bps[c][:], b_bet, t1t[:], OP.add, OP.add)
                        nc.sync.dma_start(
                            out_d[:, h0 * Wp + v0:h0 * Wp + v0 + n], ot[:])

    nc.compile()
    return nc


def _host_prep(inputs):
    x = np.asarray(inputs["x"], np.float32)
    seg = np.asarray(inputs["seg_labels"]).astype(np.int32)
    ga = float(1.0 / (1.0 + np.exp(-np.asarray(inputs["blending_gamma"], np.float64)[0])))
    ba = float(1.0 / (1.0 + np.exp(-np.asarray(inputs["blending_beta"], np.float64)[0])))
    fc_w = np.asarray(inputs["fc_w"], np.float32)
    fc_b = np.asarray(inputs["fc_b"], np.float32)
    style = np.asarray(inputs["style_codes"], np.float32)
    mu = np.maximum(np.einsum("bjd,jod->bjo", style, fc_w) + fc_b[None], 0.0)
    Rg = np.einsum("oiyx,bji->bojyx", np.asarray(inputs["conv_gamma_w"], np.float32), mu) * ga
    Rb = np.einsum("oiyx,bji->bojyx", np.asarray(inputs["conv_beta_w"], np.float32), mu) * ba
    mlp_w = np.asarray(inputs["mlp_w"], np.float32)
    iota = (np.arange(J3) % 35).astype(np.float32)[:, None]

    in_maps = []
    for core in range(8):
        b, hf = core // 2, core % 2
        sl = slice(hf * P, (hf + 1) * P)
        wseg = np.zeros((J3, 6 * P), np.float32)
        wmlp = np.zeros((J3, 3 * P), np.float32)
        for ky in range(3):
            for kx in range(3):
                rows = slice(35 * ky, 35 * ky + 35)
                wseg[rows, (kx * 2) * P:(kx * 2 + 1) * P] = Rg[b, sl, :, ky, kx].T
                wseg[rows, (kx * 2 + 1) * P:(kx * 2 + 2) * P] = Rb[b, sl, :, ky, kx].T
                wmlp[rows, kx * P:(kx + 1) * P] = mlp_w[:, :, ky, kx].T
        # fp8 DoubleRow layout: gamma pairs G0..G3 at [g*2P, g*2P+2P)
        # (cols t*P+o for the pair's two taps), single (2,2) at [8P,9P);
        # beta mirrors at +9P.
        wsp = np.zeros((P, 18 * P), np.float32)
        spg = (1 - ga) * np.asarray(inputs["sp_gamma_w"], np.float32)[sl]
        spb = (1 - ba) * np.asarray(inputs["sp_beta_w"], np.float32)[sl]
        PAIRS = [[(0, 0), (0, 1)], [(1, 0), (1, 1)], [(2, 0), (2, 1)],
                 [(0, 2), (1, 2)]]
        for g, pair in enumerate(PAIRS):
            for t, (ky, kx) in enumerate(pair):
                wsp[:, (g * 2 + t) * P:(g * 2 + t + 1) * P] = spg[:, :, ky, kx].T
                wsp[:, (9 + g * 2 + t) * P:(9 + g * 2 + t + 1) * P] = spb[:, :, ky, kx].T
        wsp[:, 8 * P:9 * P] = spg[:, :, 2, 2].T
        wsp[:, 17 * P:18 * P] = spb[:, :, 2, 2].T
        bias6 = np.zeros((P, 6), np.float32)
        bias6[:, 0] = np.asarray(inputs["mlp_b"], np.float32)
        bias6[:, 1] = (1.0 + ga * np.asarray(inputs["conv_gamma_b"], np.float32)[sl]
                       + (1 - ga) * np.asarray(inputs["sp_gamma_b"], np.float32)[sl])
        bias6[:, 2] = (ba * np.asarray(inputs["conv_beta_b"], np.float32)[sl]
                       + (1 - ba) * np.asarray(inputs["sp_beta_b"], np.float32)[sl])
        bias6[:, 3] = EPS
        seg_pad = np.full((H + 4, Wp), -1, np.int8)
        seg_pad[2:2 + H, 1:] = seg[b]
        x_pad = np.zeros((P, H, Wp), np.float32)
        x_pad[:, :, 1:] = x[b, sl]
        in_maps.append({
            "xp": np.ascontiguousarray(x_pad.reshape(P, NPIX)),
            "segp": seg_pad.reshape(-1),
            "wseg": wseg.astype(_BF16),
            "wmlp": wmlp.astype(ml_dtypes.float8_e4m3),
            "wsp": wsp.astype(ml_dtypes.float8_e4m3),
            "bias6": bias6,
            "iota1": iota,
        })
    return in_maps


def get_nc():
    global _nc_cache
    if _nc_cache is None:
        _nc_cache = _build_program()
    return _nc_cache


def kernel(**inputs):
    from concourse.bass_utils import run_bass_kernel_spmd

    in_maps = _host_prep(inputs)
    nc = get_nc()
    res = run_bass_kernel_spmd(nc, in_maps, core_ids=list(range(8)))
    out = np.empty((B, C, H, W), np.float32)
    for core in range(8):
        b, hf = core // 2, core % 2
        op = res.results[core]["out"].reshape(P, H, Wp)
        out[b, hf * P:(hf + 1) * P] = op[:, :, 1:]
    return out
